# revision 16
# baseline (speedup 1.0000x reference)
"""Single-head attention (b=4, s=4096, d=1024, h=128) on 8 Trainium2 NeuronCores.

Sharding: data-parallel over batch x query-halves -> 8 independent cores
(core c handles batch c//2, query rows [hq*2048, (hq+1)*2048) with hq = c%2).
K/V work is replicated per batch pair; no collectives.

Host prep per core: x[b].T (d-major) in bf16, packed [128, 8, 4096] with the
sequence columns rotated so the core's 2048 query rows come first (softmax
over keys is permutation-invariant so K/V key order doesn't matter).
1/sqrt(h) is folded into Wq. No max-subtraction: scores are bounded (~|6|)
so exp stays well inside bf16/fp32 range.

Device kernel (per core, everything bf16 on the PE; fp32 PSUM accum):
  qT = wq.T @ xT            [128h, 2048q]   8 accumulated d-chunk matmuls/blk
  kT|vT = wk|wv.T @ xT      [128h, 4096k]   per-block tiles
  v natural [k,h] via PE bf16 transposes of vT blocks
  per key block kb (128 keys) x q-half (1024 q), both halves' scores/exp in
  phase A interleaved with projections; h0's AV runs lag-2 behind its exp;
  h1's exp tiles are retained in SBUF and consumed by a PE-only phase B:
    scT = kT[:,kb].T @ qT   [128k, 1024q]   (PSUM, 2 matmuls)
    ex  = exp(scT)          (ACT, one [128,1024] instr, PSUM->SBUF bf16)
    dacc_h[kb%4] += ex      (DVE bf16, 4 independent sub-chains per half)
    oT_h += v[kb].T' @ ex   [128h, 1024q]   (PSUM accumulate over kb)
  den_h = ones.T @ sum(dacc_h)  (replicates the k-sum across partitions)
  outT  = oT_h * reciprocal_approx(den_h)  -> DMA out

Engine split: PE matmuls/transposes; ACT only exp (64 x [128,1024] instrs);
DVE all PSUM->SBUF copies + den sub-chains + recip + final mul.

build_nc(loop_n, timing): timing=True wraps the body in an on-device
tc.For_i loop with Internal (device-resident) inputs and a tiny external
"tick" tensor, for steady-state HW timing.
"""

import sys

sys.path.insert(0, "/opt/trn_rl_repo")

import numpy as np

import concourse.mybir as mybir
from concourse import bacc
from concourse.bass_utils import run_bass_kernel_spmd
from concourse.masks import make_identity
from concourse.tile import TileContext

F32 = mybir.dt.float32
BF = mybir.dt.bfloat16

B = 4  # batch
D = 1024  # d_model
H = 128  # head size
S = 4096  # full sequence (keys)
SQ = 2048  # queries per core
DC = D // 128  # 8 d-chunks
NB = S // 512  # 8 projection column blocks
KB = S // 128  # 32 key blocks
XT_BUFS = 4
EX_BUFS = 8


def build_nc(loop_n=0, timing=False):
    nc = bacc.Bacc("TRN2", target_bir_lowering=False, debug=False)

    in_kind = "Internal" if timing else "ExternalInput"
    xT = nc.dram_tensor("xT", (128, DC, S), BF, kind=in_kind)
    wq = nc.dram_tensor("wq", (128, DC * H), BF, kind=in_kind)
    wk = nc.dram_tensor("wk", (128, DC * H), BF, kind=in_kind)
    wv = nc.dram_tensor("wv", (128, DC * H), BF, kind=in_kind)
    outT = nc.dram_tensor("outT", (H, SQ), F32, kind="ExternalOutput")
    tick = (
        nc.dram_tensor("tick", (1, 16), F32, kind="ExternalInput") if timing else None
    )

    with TileContext(nc) as tc:
        with (
            tc.tile_pool(name="consts", bufs=1) as cpool,
            tc.tile_pool(name="big", bufs=1) as big,
            tc.tile_pool(name="xtp", bufs=XT_BUFS) as xtp,
            tc.tile_pool(name="expp", bufs=EX_BUFS) as expp,
            tc.tile_pool(name="work", bufs=2) as work,
            tc.tile_pool(name="ps", bufs=1, space="PSUM") as ps,
        ):
            # ---- constants (outside the timing loop) ----
            wq_sb = cpool.tile([128, DC * H], BF)
            nc.sync.dma_start(out=wq_sb, in_=wq[:, :])
            wk_sb = cpool.tile([128, DC * H], BF)
            nc.sync.dma_start(out=wk_sb, in_=wk[:, :])
            wv_sb = cpool.tile([128, DC * H], BF)
            nc.sync.dma_start(out=wv_sb, in_=wv[:, :])
            ones_bf = cpool.tile([128, 128], BF)
            nc.vector.memset(ones_bf, 1.0)
            ident_f = cpool.tile([128, 128], F32)
            make_identity(nc, ident_f)
            ident_bf = cpool.tile([128, 128], BF)
            nc.vector.tensor_copy(out=ident_bf, in_=ident_f)
            if timing:
                tick_sb = cpool.tile([1, 16], F32)
                nc.sync.dma_start(out=tick_sb, in_=tick[:, :])

            # ---- persistent activations ----
            q_sb = big.tile([128, SQ], BF)
            kv_sb = [big.tile([128, 1024], BF, name=f"kv{nb}") for nb in range(NB)]
            v_sb = [big.tile([128, 512], BF, name=f"v{nb}") for nb in range(NB)]
            dacc = [
                [big.tile([128, 1024], BF, name=f"dacc{h}_{j}") for j in range(4)]
                for h in range(2)
            ]

            def body(it=0):
                p = f"i{it}_"
                xts = {}

                def load_xt(nb):
                    t = xtp.tile([128, DC, 512], BF, tag="xt", name=f"{p}xt{nb}")
                    nc.sync.dma_start(out=t, in_=xT[:, :, nb * 512 : (nb + 1) * 512])
                    xts[nb] = t

                def sc_slot(name):
                    # exp-gated score slots (2 x 2 banks)
                    return ps.tile([128, 1024], F32, tag="sc", bufs=2, name=p + name)

                def pp_slot(name):
                    # projection slots, decoupled from the ACT-paced ring
                    return ps.tile([128, 512], F32, tag="pp", bufs=2, name=p + name)

                def emit_proj(nb, w_sb, out_sb, name):
                    pps = pp_slot(name)
                    for dc in range(DC):
                        nc.tensor.matmul(
                            pps,
                            w_sb[:, dc * H : (dc + 1) * H],
                            xts[nb][:, dc],
                            start=dc == 0,
                            stop=dc == DC - 1,
                        )
                    nc.vector.tensor_copy(out=out_sb, in_=pps)

                def emit_qproj(nb):
                    emit_proj(
                        nb, wq_sb, q_sb[:, nb * 512 : (nb + 1) * 512], f"qps{nb}"
                    )

                def emit_kproj(nb):
                    emit_proj(nb, wk_sb, kv_sb[nb][:, 0:512], f"kps{nb}")

                def emit_vproj(nb):
                    emit_proj(nb, wv_sb, kv_sb[nb][:, 512:1024], f"vps{nb}")

                def emit_vtransp(nb):
                    # v natural: transpose the four 128x128 bf16 blocks of vT
                    tslot = pp_slot(f"tps{nb}").bitcast(BF)
                    for t in range(4):
                        nc.tensor.transpose(
                            tslot[:, t * 128 : (t + 1) * 128],
                            kv_sb[nb][:, 512 + t * 128 : 512 + (t + 1) * 128],
                            ident_bf,
                        )
                    nc.vector.tensor_copy(out=v_sb[nb], in_=tslot[:, 0:512])

                def emit_kv(nb):
                    emit_kproj(nb)
                    emit_vproj(nb)
                    emit_vtransp(nb)

                def emit_scores_exp(kb, h):
                    nb, t = divmod(kb, 4)
                    scps = sc_slot(f"sc{kb}_{h}")
                    for c in range(2):
                        nc.tensor.matmul(
                            scps[:, c * 512 : (c + 1) * 512],
                            kv_sb[nb][:, t * 128 : (t + 1) * 128],
                            q_sb[:, h * 1024 + c * 512 : h * 1024 + (c + 1) * 512],
                            start=True,
                            stop=True,
                        )
                    if h == 0:
                        ex = expp.tile(
                            [128, 1024], BF, tag="ex0", bufs=6, name=f"{p}ex{kb}_0"
                        )
                    else:
                        # h1 exp results are retained until the AV-only phase B
                        ex = expp.tile(
                            [128, 1024], BF, tag="ex1", bufs=KB, name=f"{p}ex{kb}_1"
                        )
                    nc.scalar.activation(
                        ex, scps, mybir.ActivationFunctionType.Exp
                    )
                    # denominator: 4 independent bf16 sub-chains (j = kb % 4)
                    d = dacc[h][kb % 4]
                    if kb < 4:
                        nc.vector.tensor_copy(out=d, in_=ex)
                    else:
                        nc.vector.tensor_add(d, d, ex)
                    return ex

                def emit_av(kb, ex, oT):
                    nb, t = divmod(kb, 4)
                    for c in range(2):
                        cc = slice(c * 512, (c + 1) * 512)
                        nc.tensor.matmul(
                            oT[:, cc],
                            v_sb[nb][:, t * 128 : (t + 1) * 128],
                            ex[:, cc],
                            start=kb == 0,
                            stop=kb == KB - 1,
                        )

                def emit_den_recip(h):
                    d = dacc[h]
                    nc.vector.tensor_add(d[0], d[0], d[1])
                    nc.vector.tensor_add(d[2], d[2], d[3])
                    nc.vector.tensor_add(d[0], d[0], d[2])
                    recips = []
                    for c in range(2):
                        cc = slice(c * 512, (c + 1) * 512)
                        dslot = pp_slot(f"den{h}_{c}")
                        nc.tensor.matmul(
                            dslot, ones_bf, dacc[h][0][:, cc], start=True, stop=True
                        )
                        recip = work.tile(
                            [128, 512], F32, tag="recip", name=f"{p}rc{h}_{c}"
                        )
                        nc.vector.reciprocal_approx_fast(out=recip, in_=dslot)
                        recips.append(recip)
                    return recips

                def emit_mul_out(h, oT, recips):
                    for c in range(2):
                        cc = slice(c * 512, (c + 1) * 512)
                        onrm = work.tile(
                            [128, 512], F32, tag="onrm", name=f"{p}on{h}_{c}"
                        )
                        nc.vector.tensor_mul(onrm, oT[:, cc], recips[c])
                        nc.sync.dma_start(
                            out=outT[:, h * 1024 + c * 512 : h * 1024 + (c + 1) * 512],
                            in_=onrm,
                        )

                # ---- emission schedule ----
                # Phase A: projections + both halves' scores/exp + h0's AV.
                # Phase B: h1's AV (pure PE) from retained ex tiles.
                for nb in range(4):
                    load_xt(nb)
                emit_qproj(0)
                emit_kv(0)
                emit_qproj(1)
                emit_qproj(2)
                emit_qproj(3)
                oT0 = ps.tile([128, 1024], F32, tag="oT", bufs=1, name=f"{p}oT0")
                ex1s = {}
                pend0 = []
                for kb in range(KB):
                    nb, t = divmod(kb, 4)
                    ex0 = emit_scores_exp(kb, 0)
                    if len(pend0) >= 2:
                        emit_av(*pend0.pop(0), oT0)
                    pend0.append((kb, ex0))
                    ex1s[kb] = emit_scores_exp(kb, 1)
                    # spread next block's projection work across the group
                    if t == 0 and nb > 0:
                        emit_vtransp(nb)
                    if nb + 1 < NB:
                        if t == 1:
                            emit_kproj(nb + 1)
                        elif t == 2:
                            if nb + 4 < NB:
                                load_xt(nb + 4)
                        elif t == 3:
                            emit_vproj(nb + 1)
                recips0 = emit_den_recip(0)
                for kb_, ex_ in pend0:
                    emit_av(kb_, ex_, oT0)
                emit_mul_out(0, oT0, recips0)
                # h1's oT lives in the sc ring, which is free during phase B.
                # c-major order so chunk 0's normalize+DMA overlaps chunk 1's AV.
                oT1 = sc_slot("oT1")
                recips1 = emit_den_recip(1)
                for c in range(2):
                    cc = slice(c * 512, (c + 1) * 512)
                    for kb in range(KB):
                        nb, t = divmod(kb, 4)
                        nc.tensor.matmul(
                            oT1[:, cc],
                            v_sb[nb][:, t * 128 : (t + 1) * 128],
                            ex1s[kb][:, cc],
                            start=kb == 0,
                            stop=kb == KB - 1,
                        )
                    onrm = work.tile([128, 512], F32, tag="onrm", name=f"{p}on1_{c}")
                    nc.vector.tensor_mul(onrm, oT1[:, cc], recips1[c])
                    nc.sync.dma_start(
                        out=outT[:, 1024 + c * 512 : 1024 + (c + 1) * 512], in_=onrm
                    )

            if timing:
                with tc.For_i(0, loop_n):
                    body()
            else:
                body()

    nc.compile()
    return nc


_NC_CACHE = None


def _get_nc():
    global _NC_CACHE
    if _NC_CACHE is None:
        _NC_CACHE = build_nc()
    return _NC_CACHE


def _pack_dmajor(a):
    # [1024, n] -> [128, 8, n] with row r = dc*128 + p
    import ml_dtypes

    n = a.shape[1]
    return np.ascontiguousarray(
        a.reshape(DC, 128, n).transpose(1, 0, 2).astype(ml_dtypes.bfloat16)
    )


def kernel(x, Wq, Wk, Wv):
    x = np.asarray(x, dtype=np.float32)
    Wq = np.asarray(Wq, dtype=np.float32)
    Wk = np.asarray(Wk, dtype=np.float32)
    Wv = np.asarray(Wv, dtype=np.float32)
    assert x.shape == (B, S, D), x.shape

    wq = _pack_dmajor(Wq / np.sqrt(np.float32(H)))
    wk = _pack_dmajor(Wk)
    wv = _pack_dmajor(Wv)
    in_maps = []
    for c in range(8):
        bi, hq = divmod(c, 2)
        xt = x[bi].T  # [d, s]
        if hq == 1:
            xt = np.concatenate([xt[:, SQ:], xt[:, :SQ]], axis=1)
        in_maps.append({"xT": _pack_dmajor(xt), "wq": wq, "wk": wk, "wv": wv})

    nc = _get_nc()
    res = run_bass_kernel_spmd(nc, in_maps, core_ids=list(range(8)))

    out = np.empty((B, S, H), dtype=np.float32)
    for c in range(8):
        bi, hq = divmod(c, 2)
        out[bi, hq * SQ : (hq + 1) * SQ] = res.results[c]["outT"].T
    return out


if __name__ == "__main__":
    rng = np.random.default_rng(0)
    x = rng.standard_normal((B, S, D), dtype=np.float32)
    s = 1.0 / np.sqrt(D)
    Wq = rng.standard_normal((D, H), dtype=np.float32) * s
    Wk = rng.standard_normal((D, H), dtype=np.float32) * s
    Wv = rng.standard_normal((D, H), dtype=np.float32) * s
    out = kernel(x=x, Wq=Wq, Wk=Wk, Wv=Wv)
    print("out", out.shape, out.dtype, float(np.abs(out).max()))
